# revision 2
# baseline (speedup 1.0000x reference)
"""Causal self-attention (RoPE, 16 heads, B=2 T=2048 C=1024) on 8 TRN2 cores, v2.

Sharding: core = b*4 + g  (b = batch, g = head-group of 4 heads); host sums
the 4 w_proj partials per batch.

v2 design notes (vs v1):
- RoPE rotate-half is a PE matmul against a signed permutation matrix
  (host-built) instead of 4 partition-swap DMAs per slice: the Sync queue
  drops ~40us of traffic and the serial RoPE tail that idled the PE >3.4us
  at the qkv->attention boundary (which re-throttled the HAM clock gate to
  1.2 GHz for the whole attention phase) is gone.
- Warmup matmuls during the initial xT DMA wait get the PE to K=8/8 early;
  the emission order keeps every PE idle gap well under the ~3.4us HAM
  window so the whole kernel runs at 2.4 GHz.
- Attention uses 512-wide q slices; both heads of a pair score into ONE
  [128,1024] PSUM tile so a single ACT exp instruction (3D AP, trimmed to
  the causal region) covers the pair: 80 exps instead of 192 -> ACT (the
  attention bottleneck at 0.83ns/col) stays ~25us under the PE total.
- The m3/m1 qkv matmuls + their RoPE are deferred into a job queue pumped
  between pair-0 attention units (exp-paced PE slack); proj matmuls pump
  into pair-1's stream the same way. y matmuls for pair 0 are backlogged
  until the qkv PSUM pools retire (PSUM: sps 4 + yps 2 + proj 2 = 8 banks).
"""

import numpy as np
from collections import deque
from functools import partial

B = 2
T = 2048
C = 1024
N_HEAD = 16
HD = 64
HPC = 4           # heads per core
N_CORES = 8
ROPE_BASE = 10000.0
TS = 512          # qkv t-slice width
QW = 512          # attention q-slice width
VW = HD + 1       # v_ext per-head width (v + ones column)

_CACHE = {}


def _np_bf16():
    import ml_dtypes
    return np.dtype(ml_dtypes.bfloat16)


def _build(t_len=T, debug=False):
    import concourse.tile as tile
    from concourse import bacc, mybir

    F32 = mybir.dt.float32
    BF16 = mybir.dt.bfloat16

    n_ts = t_len // TS          # qkv t-slices (4)
    n_tt = t_len // 128         # 128-row t-tiles (16)
    n_jj = t_len // QW          # attention q-slices (4)

    nc = bacc.Bacc(None, target_bir_lowering=False, debug=False)
    with tile.TileContext(nc) as tc:
        with tc.tile_pool(name="dram", bufs=1, space="DRAM") as dram:
            xT = dram.tile([C, t_len], BF16, kind="ExternalInput")
            wqk = dram.tile([C, 4 * 128], BF16, kind="ExternalInput")
            wv = dram.tile([C, 4 * HD], BF16, kind="ExternalInput")
            wo = dram.tile([4 * HD, C], BF16, kind="ExternalInput")
            cosb = dram.tile([128, t_len], BF16, kind="ExternalInput")
            sinf = dram.tile([128, t_len], F32, kind="ExternalInput")
            utri = dram.tile([128, 128], BF16, kind="ExternalInput")
            psgn = dram.tile([128, 128], BF16, kind="ExternalInput")
            ones4 = dram.tile([128, n_tt * HPC], BF16, kind="ExternalInput")
            out = dram.tile([t_len, C], F32, kind="ExternalOutput")
            dbg = {}
            if debug:
                for m in range(4):
                    dbg[f"qkT{m}"] = dram.tile([128, t_len], F32,
                                               kind="ExternalOutput",
                                               name=f"dbg_qkT{m}")
                for k in range(2):
                    dbg[f"yT{k}"] = dram.tile([128, t_len], F32,
                                              kind="ExternalOutput",
                                              name=f"dbg_yT{k}")

            xT_c = xT.rearrange("(a p) t -> a p t", p=128)    # [8, 128, T]
            wqk_c = wqk.rearrange("(a p) m -> a p m", p=128)  # [8, 128, 512]
            wv_c = wv.rearrange("(a p) m -> a p m", p=128)    # [8, 128, 256]
            wo_c = wo.rearrange("(a p) m -> a p m", p=128)    # [2, 128, 1024]

            with (
                tc.tile_pool(name="persist", bufs=1) as persist,
                tc.tile_pool(name="rope_pool", bufs=3) as rope_pool,
                tc.tile_pool(name="px_pool", bufs=30) as px_pool,
                tc.tile_pool(name="n_pool", bufs=3) as n_pool,
                tc.tile_pool(name="osb_pool", bufs=3) as osb_pool,
            ):
                qkT = [persist.tile([128, t_len], BF16, name=f"qkT{m}")
                       for m in range(4)]
                # v_ext layout [128, n_tt, HPC, VW]: per head cols 0..63 = v,
                # col 64 = ones (softmax denominator on PSUM partition 64)
                vext_sb = persist.tile([128, n_tt * HPC * VW], BF16)
                vext_v = vext_sb.rearrange("p (i h d) -> p i h d",
                                           i=n_tt, d=VW)
                yT = [persist.tile([128, t_len], BF16, name=f"yT{k}")
                      for k in range(2)]
                cos_sb = persist.tile([128, t_len], BF16)
                sin_sb = persist.tile([128, t_len], F32)
                utri_sb = persist.tile([128, 128], BF16)
                psgn_sb = persist.tile([128, 128], BF16)
                wqk_sb = [persist.tile([128, 4 * 128], BF16, name=f"wqk{c}")
                          for c in range(8)]
                wv_sb = [persist.tile([128, 4 * HD], BF16, name=f"wv{c}")
                         for c in range(8)]
                wo_sb = [persist.tile([128, C], BF16, name=f"wo{k}")
                         for k in range(2)]
                xT_sb = [persist.tile([128, t_len], BF16, name=f"xTsb{c}")
                         for c in range(8)]

                # ---- DMAs (scalar HWDGE: consts+weights+tables;
                #      sync HWDGE: xT stream; gpsimd SWDGE: ones) ----
                nc.scalar.dma_start(out=utri_sb, in_=utri[:])
                nc.scalar.dma_start(out=psgn_sb, in_=psgn[:])
                nc.gpsimd.dma_start(
                    out=vext_v[:, :, :, HD:],
                    in_=ones4[:].rearrange("p (i h o) -> p i h o",
                                           i=n_tt, o=1),
                )
                for c in range(8):
                    nc.scalar.dma_start(out=wqk_sb[c], in_=wqk_c[c])
                for c in range(8):
                    nc.sync.dma_start(out=xT_sb[c], in_=xT_c[c])
                for c in range(8):
                    nc.scalar.dma_start(out=wv_sb[c], in_=wv_c[c])
                nc.scalar.dma_start(out=cos_sb, in_=cosb[:])
                nc.scalar.dma_start(out=sin_sb, in_=sinf[:])
                for k in range(2):
                    nc.scalar.dma_start(out=wo_sb[k], in_=wo_c[k])

                jobs = deque()

                def pump(k=1):
                    for _ in range(k):
                        if jobs:
                            jobs.popleft()()

                # PSUM pools with manual lifetime control
                sw_ps = tc.alloc_tile_pool(name="sw_ps", bufs=2, space="PSUM")
                qkB_ps = tc.alloc_tile_pool(name="qkB_ps", bufs=2,
                                            space="PSUM")

                def rope(m, ts, qkps):
                    """qkT[m][:, slice] = qkps*cos + (Psgn@qkps)*sin."""
                    t0 = ts * TS
                    qksb = rope_pool.tile([128, TS], BF16, tag="qksb",
                                          name=f"qksb_{m}_{ts}")
                    nc.scalar.copy(out=qksb, in_=qkps[:])
                    t1 = rope_pool.tile([128, TS], F32, tag="t1",
                                        name=f"t1_{m}_{ts}")
                    nc.vector.tensor_mul(t1, qksb, cos_sb[:, t0:t0 + TS])
                    swp = sw_ps.tile([128, TS], F32, tag="sw",
                                     name=f"sw_{m}_{ts}")
                    nc.tensor.matmul(out=swp, lhsT=psgn_sb, rhs=qksb,
                                     start=True, stop=True)
                    t2 = rope_pool.tile([128, TS], F32, tag="t2",
                                        name=f"t2_{m}_{ts}")
                    nc.vector.tensor_mul(t2, swp, sin_sb[:, t0:t0 + TS])
                    nc.vector.tensor_add(qkT[m][:, t0:t0 + TS], t1, t2)

                # ---- m2 (k01) c-major: paced by the xT DMA stream ----
                with tc.tile_pool(name="qkA_ps", bufs=4,
                                  space="PSUM") as qkA_ps:
                    # warmup: PE busy during DMA wait -> HAM warms early
                    wps = qkB_ps.tile([128, TS], F32, tag="qk", name="warm")
                    for _ in range(16):
                        nc.tensor.matmul(out=wps[:, :128], lhsT=utri_sb,
                                         rhs=utri_sb, start=True, stop=True)
                    m = 2
                    accs2 = [qkA_ps.tile([128, TS], F32, tag="acc",
                                         name=f"m2acc{ts}")
                             for ts in range(n_ts)]
                    for c in range(8):
                        for ts in range(n_ts):
                            nc.tensor.matmul(
                                out=accs2[ts],
                                lhsT=wqk_sb[c][:, m * 128:(m + 1) * 128],
                                rhs=xT_sb[c][:, ts * TS:(ts + 1) * TS],
                                start=(c == 0), stop=(c == 7),
                            )
                    for ts in range(n_ts):
                        jobs.append(partial(rope, 2, ts, accs2[ts]))

                    # ---- m0 (q01) ts-major; m2 ropes pumped between ----
                    m = 0
                    for ts in range(n_ts):
                        acc = qkB_ps.tile([128, TS], F32, tag="qk",
                                          name=f"m0acc{ts}")
                        for c in range(8):
                            nc.tensor.matmul(
                                out=acc,
                                lhsT=wqk_sb[c][:, 0:128],
                                rhs=xT_sb[c][:, ts * TS:(ts + 1) * TS],
                                start=(c == 0), stop=(c == 7),
                            )
                        jobs.append(partial(rope, 0, ts, acc))
                        pump(1)

                # ---- v phase (i-major); m0 ropes pumped between ----
                with tc.tile_pool(name="v_ps", bufs=3,
                                  space="PSUM") as v_ps:
                    for i in range(n_tt):
                        vps = v_ps.tile([128, TS], F32, tag="v",
                                        name=f"vps{i}")
                        for c in range(8):
                            nc.tensor.matmul(
                                out=vps[:, :4 * HD],
                                lhsT=xT_sb[c][:, i * 128:(i + 1) * 128],
                                rhs=wv_sb[c][:],
                                start=(c == 0), stop=(c == 7),
                            )
                        nc.vector.tensor_copy(
                            out=vext_v[:, i, :, :HD],
                            in_=vps[:, :4 * HD].rearrange(
                                "p (h d) -> p h d", d=HD),
                        )
                        if i % 2 == 1:
                            pump(1)

                # ---- m3/m1 (k23, q23) as deferred MM-pair jobs ----
                qk_live = {}

                def qk_mm(m, ts, c0):
                    key = (m, ts)
                    if key not in qk_live:
                        qk_live[key] = qkB_ps.tile([128, TS], F32, tag="qk",
                                                   name=f"acc{m}_{ts}")
                    acc = qk_live[key]
                    for c in (c0, c0 + 1):
                        nc.tensor.matmul(
                            out=acc,
                            lhsT=wqk_sb[c][:, m * 128:(m + 1) * 128],
                            rhs=xT_sb[c][:, ts * TS:(ts + 1) * TS],
                            start=(c == 0), stop=(c == 7),
                        )
                    if c0 == 6:
                        jobs.appendleft(partial(rope, m, ts, acc))

                for m in (3, 1):
                    for ts in range(n_ts):
                        for c0 in (0, 2, 4, 6):
                            jobs.append(partial(qk_mm, m, ts, c0))

                # ---------------- attention ----------------
                sps_pool = tc.alloc_tile_pool(name="sps_ps", bufs=2,
                                              space="PSUM", side="right")

                def emit_unit(pair, jj, i):
                    """Scores for both heads of `pair` into one PSUM tile,
                    one pair-exp, diagonal utri mask. Returns (px, off)."""
                    base = jj * QW
                    off = max(0, 128 * i - base)
                    w = QW - off
                    q_t, k_t = qkT[pair], qkT[2 + pair]
                    sps = sps_pool.tile([128, 2 * QW], F32, tag="sps",
                                        name=f"sps{pair}_{jj}_{i}")
                    for h2 in (0, 1):
                        hoff = 64 * h2
                        nc.tensor.matmul(
                            out=sps[:, h2 * QW + off:(h2 + 1) * QW],
                            lhsT=k_t[hoff:hoff + 64, 128 * i:128 * (i + 1)],
                            rhs=q_t[hoff:hoff + 64, base + off:base + QW],
                            start=True, stop=True,
                        )
                    px = px_pool.tile([128, 2 * QW], BF16, tag="px",
                                      name=f"px{pair}_{jj}_{i}")
                    s3 = sps.rearrange("p (h w) -> p h w", h=2)
                    p3 = px.rearrange("p (h w) -> p h w", h=2)
                    nc.scalar.activation(
                        out=p3[:, :, off:], in_=s3[:, :, off:],
                        func=mybir.ActivationFunctionType.Exp,
                    )
                    if i >= 4 * jj:     # diagonal block: mask with utri
                        for h2 in (0, 1):
                            sl = px[:, h2 * QW + off:h2 * QW + off + 128]
                            nc.vector.tensor_mul(sl, sl, utri_sb)
                    return px, off

                def emit_y(pair, jj, i, px, off, yps2):
                    for h2 in (0, 1):
                        h = pair * 2 + h2
                        vcol = (i * HPC + h) * VW
                        nc.tensor.matmul(
                            out=yps2[h2][:, off:QW],
                            lhsT=vext_sb[:, vcol:vcol + VW],
                            rhs=px[:, h2 * QW + off:(h2 + 1) * QW],
                            start=(i == 0), stop=(i == 4 * jj + 3),
                        )

                def norm(pair, jj, h2, yps, fast=False):
                    """yT slice = yps rows / softmax denominator row.
                    fast=True: reciprocal directly on the [1,QW] row (skips
                    the strip DMA round-trip) — used on the drain critical
                    path where latency matters more than DVE time."""
                    h = pair * 2 + h2
                    base = jj * QW
                    hoff = 64 * (h % 2)
                    ycp = n_pool.tile([VW, QW], F32, tag="ycp",
                                      name=f"ycp_{h}_{jj}")
                    nc.vector.tensor_copy(out=ycp, in_=yps[:])
                    rrow = n_pool.tile([1, QW], F32, tag="rrow",
                                       name=f"rrow_{h}_{jj}")
                    strip = n_pool.tile([4, 128], F32, tag="strip",
                                        name=f"strip_{h}_{jj}")
                    nc.sync.dma_start(
                        out=strip,
                        in_=ycp[HD:VW, :].rearrange("p (a b) -> p a b",
                                                    b=128))
                    rstrip = n_pool.tile([4, 128], F32, tag="rstrip",
                                         name=f"rstrip_{h}_{jj}")
                    nc.vector.reciprocal_approx_fast(out=rstrip,
                                                     in_=strip)
                    nc.sync.dma_start(
                        out=rrow.rearrange("p (a b) -> p a b", b=128),
                        in_=rstrip)
                    bcast = n_pool.tile([HD, QW], F32, tag="bcast",
                                        name=f"bcast_{h}_{jj}")
                    nc.gpsimd.partition_broadcast(bcast[:], rrow[:])
                    nout = n_pool.tile([HD, QW], BF16, tag="nout",
                                       name=f"nout_{h}_{jj}")
                    nc.vector.tensor_mul(nout, ycp[:HD, :], bcast)
                    # drain-critical last norms go via the idle ACT queue
                    dq = nc.scalar if fast else nc.sync
                    dq.dma_start(
                        out=yT[h // 2][hoff:hoff + HD, base:base + QW],
                        in_=nout,
                    )

                # --- pair 0, all jj: s/exp stream; qkv jobs pumped in ---
                p0_px = {}
                yps_of = {}
                for jj in range(n_jj):
                    for i in range(4 * jj + 4):
                        p0_px[(jj, i)] = emit_unit(0, jj, i)
                        pump(2)
                        if jj == 3 and not jobs:
                            break
                    if jj == 3:
                        break
                while jobs:
                    pump(1)

                # qkv PSUM pools retire; attention y/proj pools open
                qkB_ps.release()
                sw_ps.release()
                yps_pool = tc.alloc_tile_pool(name="yps_ps", bufs=2,
                                              space="PSUM", side="right")
                o_ps = tc.alloc_tile_pool(name="o_ps", bufs=2, space="PSUM",
                                          side="right")

                def open_yps(pair, jj):
                    yps2 = [yps_pool.tile([VW, QW], F32, tag="yps",
                                          name=f"yps{pair}_{jj}_{h2}")
                            for h2 in (0, 1)]
                    yps_of[(pair, jj)] = yps2
                    return yps2

                def proj_mm(tt, cs):
                    ops = o_ps.tile([128, 512], F32, tag="o",
                                    name=f"o{tt}_{cs}")
                    for k in (0, 1):
                        nc.tensor.matmul(
                            out=ops,
                            lhsT=yT[k][:, tt * 128:(tt + 1) * 128],
                            rhs=wo_sb[k][:, cs * 512:(cs + 1) * 512],
                            start=(k == 0), stop=(k == 1),
                        )
                    osb = osb_pool.tile([128, 512], F32, tag="osb",
                                        name=f"osb_{tt}_{cs}")
                    nc.vector.tensor_copy(out=osb, in_=ops)
                    dma = nc.sync if cs == 0 else nc.scalar
                    dma.dma_start(
                        out=out[tt * 128:(tt + 1) * 128,
                                cs * 512:(cs + 1) * 512],
                        in_=osb,
                    )

                # y backlog for pair0 jj0..2 (+norms), interleaved with the
                # remaining pair0 jj3 units
                remaining_p03 = [i for i in range(16)
                                 if (3, i) not in p0_px]
                backlog = deque()
                for jj in range(3):
                    yps2 = open_yps(0, jj)
                    for i in range(4 * jj + 4):
                        px, off = p0_px[(jj, i)]
                        backlog.append(partial(emit_y, 0, jj, i, px, off,
                                               yps2))
                    backlog.append(partial(norm, 0, jj, 0, yps2[0]))
                    backlog.append(partial(norm, 0, jj, 1, yps2[1]))

                def pump_b(k=1):
                    for _ in range(k):
                        if backlog:
                            backlog.popleft()()

                done_p03 = [i for i in range(16) if (3, i) in p0_px]
                for i in done_p03:
                    pump_b(2)
                for i in remaining_p03:
                    p0_px[(3, i)] = emit_unit(0, 3, i)
                    pump_b(2)
                while backlog:
                    pump_b(1)

                # pair0 jj3 y + norms become pumped jobs for pair1's stream
                yps2_03 = open_yps(0, 3)
                for i in range(16):
                    px, off = p0_px[(3, i)]
                    jobs.append(partial(emit_y, 0, 3, i, px, off, yps2_03))
                jobs.append(partial(norm, 0, 3, 0, yps2_03[0]))
                jobs.append(partial(norm, 0, 3, 1, yps2_03[1]))

                # --- pair 1: s/exp stream; ALL deferred PE work (pair0-jj3
                # y+norms, pair1 y+norms, proj) flows through the single
                # FIFO so emission order matches pool-slot release order ---
                for jj in range(n_jj):
                    yps2 = open_yps(1, jj)
                    for i in range(4 * jj + 4):
                        px, off = emit_unit(1, jj, i)
                        jobs.append(partial(emit_y, 1, jj, i, px, off, yps2))
                        pump(2 if jj < 3 else 3)
                    lastn = (jj == 3)
                    jobs.append(partial(norm, 1, jj, 0, yps2[0],
                                        fast=lastn))
                    jobs.append(partial(norm, 1, jj, 1, yps2[1],
                                        fast=lastn))
                    # proj for this q range: needs all 4 heads' norms at jj
                    for tt in range(4 * jj, 4 * jj + 4):
                        for cs in (0, 1):
                            jobs.append(partial(proj_mm, tt, cs))

                while jobs:
                    pump(1)

                o_ps.release()
                yps_pool.release()
                sps_pool.release()

                if debug:
                    for m in range(4):
                        nc.sync.dma_start(out=dbg[f"qkT{m}"][:],
                                          in_=qkT[m][:].bitcast(F32))
                    for k in range(2):
                        nc.sync.dma_start(out=dbg[f"yT{k}"][:],
                                          in_=yT[k][:].bitcast(F32))

    nc.compile()
    names = dict(
        xT=xT.name, wqk=wqk.name, wv=wv.name, wo=wo.name,
        cosb=cosb.name, sinf=sinf.name, utri=utri.name, psgn=psgn.name,
        ones4=ones4.name, out=out.name,
    )
    for k, v in dbg.items():
        names["dbg_" + k] = v.name
    return nc, names


# Head-dim permutation: evens first, odds last — turns the interleaved
# rotate-half pair swap into a contiguous 32-row block swap on device.
PERM = np.concatenate([np.arange(0, HD, 2), np.arange(1, HD, 2)])


def _host_constants(t_len=T):
    bf16 = _np_bf16()
    inv_freq = 1.0 / (ROPE_BASE ** (np.arange(0, HD, 2, dtype=np.float64) / HD))
    t = np.arange(t_len, dtype=np.float64)
    freqs = np.outer(t, inv_freq)                      # [T, 32]
    emb = np.concatenate([freqs, freqs], axis=-1)      # [T, 64]
    cosT = np.cos(emb).T.astype(np.float32)            # [64, T]
    sinT = np.sin(emb).T.astype(np.float32)
    cosP, sinP = cosT[PERM], sinT[PERM]
    cos128 = np.vstack([cosP, cosP]).astype(bf16)      # [128, T]
    sin128 = np.vstack([sinP, sinP]).astype(np.float32).copy()
    utri = np.triu(np.ones((128, 128), dtype=np.float32)).astype(bf16)
    # signed rotate-half permutation: out[hb+j] = -in[hb+32+j],
    # out[hb+32+j] = +in[hb+j]  (lhsT[k, m] = coeff of in[k] in out[m])
    psgn = np.zeros((128, 128), dtype=np.float32)
    for hb in (0, 64):
        for j in range(32):
            psgn[hb + 32 + j, hb + j] = -1.0
            psgn[hb + j, hb + 32 + j] = 1.0
    psgn = psgn.astype(bf16)
    ones4 = np.ones((128, (t_len // 128) * HPC), dtype=np.float32).astype(bf16)
    return cos128, sin128, utri, psgn, ones4


def _perm_heads(w):
    """Permute each head's 64 columns of w [C, HPC*HD] by PERM."""
    Cdim = w.shape[0]
    return w.reshape(Cdim, HPC, HD)[:, :, PERM].reshape(Cdim, HPC * HD)


def _core_inputs(x, w_attn, w_proj, t_len=T):
    """Build the per-core input maps (values only, keyed by logical name)."""
    bf16 = _np_bf16()
    cos128, sin128, utri, psgn, ones4 = _host_constants(t_len)
    per_core = []
    for core in range(N_CORES):
        b, g = divmod(core, 4)
        h0 = g * HPC * HD                       # column offset of first head
        wq = _perm_heads(w_attn[:, h0:h0 + HPC * HD])
        wk = _perm_heads(w_attn[:, C + h0:C + h0 + HPC * HD]
                         * np.float32(1.0 / np.sqrt(HD)))
        wvs = w_attn[:, 2 * C + h0:2 * C + h0 + HPC * HD]
        per_core.append(dict(
            xT=np.ascontiguousarray(x[b].T).astype(bf16),
            wqk=np.ascontiguousarray(
                np.concatenate([wq, wk], axis=1)).astype(bf16),
            wv=np.ascontiguousarray(wvs).astype(bf16),
            wo=np.ascontiguousarray(w_proj[h0:h0 + HPC * HD, :]).astype(bf16),
            cosb=cos128, sinf=sin128, utri=utri, psgn=psgn, ones4=ones4,
        ))
    return per_core


def kernel(x, w_attn, w_proj):
    from concourse.bass_utils import run_bass_kernel_spmd

    x = np.asarray(x, dtype=np.float32)
    w_attn = np.asarray(w_attn, dtype=np.float32)
    w_proj = np.asarray(w_proj, dtype=np.float32)

    if "nc" not in _CACHE:
        _CACHE["nc"], _CACHE["names"] = _build(T)
    nc, names = _CACHE["nc"], _CACHE["names"]

    per_core = _core_inputs(x, w_attn, w_proj, T)
    in_maps = [{names[k]: v for k, v in m.items()} for m in per_core]
    r = run_bass_kernel_spmd(nc, in_maps, core_ids=list(range(N_CORES)))

    full = np.zeros((B, T, C), dtype=np.float64)
    for core in range(N_CORES):
        full[core // 4] += r.results[core][names["out"]].astype(np.float64)
    return full.astype(np.float32)
